# revision 14
# baseline (speedup 1.0000x reference)
"""Trainium2 Bass kernel for blocked-DCT high-frequency extractor.

Computes, for x (64, 3, 512, 512) f32:
  gray = 0.299*R + 0.587*G + 0.114*B                     (B,1,H,W)
  per 8x8 block:  Y = mask * (D @ block @ D.T)           (2D DCT + high-pass)
  output (64, 1, 512, 512) f32

Strategy (pure data parallel over batch, 8 batches/core on 8 cores; the
kernel is HBM-bound: 24 MiB in + 8 MiB out per core; DMA-union busy on
HW measures ~94.4 us at ~355 GB/s = the per-NC HBM cap, so the span
target is startup + 94.4 us + a short drain).

Per core, per (batch, 128-row chunk) of the image:
  1. One fused 768 KB DMA on the SP HWDGE queue brings all 3 channel
     chunks into a (128h, 3*512w) tile (2 KB contiguous runs).
  2. H-direction DCT *and* grayscale in one PSUM accumulation group:
     three float32r matmuls (one per channel c) with stationary weight
     w_c * (I_16 kron D^T) summing into one PSUM tile, written out as
     bf16. float32r streams fp32 data through the PE at 1 cycle/row
     (vs 4 for plain f32) when the moving free dim is >= 256. This
     removes the whole 3-engine grayscale stage of the earlier version
     (DVE stt + ACT mul + GpSimd add, ~2.7 us/chunk combined).
  3. DVE stream-transpose (independent 32x32 blocks) straight out of
     PSUM, 2-byte dtype for double DVE throughput. Because 8 | 32, this
     puts w%32 (which contains the intra-block w index) on partitions.
  4. W-direction DCT *and* high-pass mask as two bf16 matmuls over the
     two strided free-column groups f%8<4 / f%8>=4: the mask only
     depends on (out-partition % 8 < 4) && (free % 8 < 4), so the f<4
     group uses weights whose columns are pre-scaled by the 0/1 mask
     vector and the f>=4 group uses plain weights. The mask costs zero
     extra ops anywhere.
  5. DVE stream-transpose back, f32 PSUM -> SBUF -> exact natural
     row-major output layout. ACT runs no compute at all - it only
     issues the output DMAs (and the const loads) on its HWDGE queue.
  6. 256 KB contiguous output DMA on the ACT HWDGE queue (separate
     queue from the SP input stream).

The 32x32 block transpose is an involution whose block-nesting (8 | 32)
makes both DCT matmuls use the same I_16 kron D^T stationary weight
(mm1's copies scaled by the grayscale weights, mm2's masked copy by the
mask vector) and lands the final result in natural row-major layout
with zero TensorE transposes.
"""

import os

import numpy as np

import concourse.bacc as bacc
import concourse.mybir as mybir
import concourse.tile as tile
from concourse.bass_utils import run_bass_kernel_spmd

N_CORES = 8
B, C, H, W = 64, 3, 512, 512
BLOC = B // N_CORES  # batches per core
P = 128              # SBUF partitions / chunk height
NCH = H // P         # 128-row chunks per image
F32 = mybir.dt.float32
F32R = mybir.dt.float32r
BF16 = mybir.dt.bfloat16
GRAY_W = (0.299, 0.587, 0.114)

_NC = None          # cached compiled Bass module
LAST_RUN = None     # BassKernelResults of the most recent run (for test.py)


def _build_bass():
    nc = bacc.Bacc(
        "TRN2",
        target_bir_lowering=False,
        debug=False,
        num_devices=N_CORES,
    )
    x = nc.declare_dram_parameter("x", [BLOC, C, H, W], F32R, isOutput=False)
    # wts[:, 3*128]: [w0*K | w1*K | w2*K] with K = I_16 kron D^T (mm1, f32r)
    wts = nc.declare_dram_parameter("wts", [P, 3 * P], F32R, isOutput=False)
    # bf16 weights for the W-direction matmul: [mask-scaled K | plain K]
    wtsb = nc.declare_dram_parameter("wtsb", [P, 2 * P], BF16, isOutput=False)
    out = nc.declare_dram_parameter("out", [BLOC, 1, H, W], F32, isOutput=True)

    with tile.TileContext(nc) as tc:
        with (
            tc.tile_pool(name="consts", bufs=1) as consts,
            tc.tile_pool(name="xin", bufs=8) as xin,
            tc.tile_pool(name="work", bufs=6) as work,
            tc.tile_pool(name="psum", bufs=4, space="PSUM") as psum_pool,
        ):
            # consts ride the ACT HWDGE queue so the SP queue's first
            # descriptor batch is already the first 768 KB input chunk.
            wd = consts.tile([P, 3 * P], F32R, tag="wd")
            nc.scalar.dma_start(wd[:], wts[:])
            wdb = consts.tile([P, 2 * P], BF16, tag="wdb")
            nc.scalar.dma_start(wdb[:], wtsb[:])

            # out-DMA for chunk i is emitted at the top of iteration i+1 so
            # ACT's in-order stream never parks on the wait for DVE's final
            # transpose ahead of the next chunk's compute ops.
            pending = None
            for b in range(BLOC):
                for hc in range(NCH):
                    hs = hc * P
                    # one 768 KB DMA: channels side by side in the free dim
                    xt = xin.tile([P, C * W], F32R, tag="x")
                    xsrc = x[b].rearrange("c (n p) w -> n p c w", p=P)[hc]
                    nc.sync.dma_start(
                        xt[:].rearrange("p (c w) -> p c w", w=W), xsrc
                    )
                    # H-direction DCT + grayscale: 3 accumulating f32r
                    # matmuls, one per channel; PSUM tile read back as bf16
                    p1 = psum_pool.tile([P, W], F32, tag="p1")
                    for c in range(C):
                        nc.tensor.matmul(
                            p1[:],
                            wd[:, c * P:(c + 1) * P],
                            xt[:, c * W:(c + 1) * W],
                            start=(c == 0),
                            stop=(c == C - 1),
                        )
                    # delayed out-DMA: its dep (prev chunk's transpose) is
                    # long done by the time ACT drains to it
                    if pending is not None:
                        nc.scalar.dma_start(*pending)
                    # PSUM -> SBUF move with bf16 rounding on ACT (its
                    # only compute op), then a 2-byte 32x32 block transpose
                    s1b = work.tile([P, W], BF16, tag="s1b")
                    nc.scalar.copy(s1b[:], p1[:])
                    s1t = work.tile([P, W], BF16, tag="s1t")
                    nc.vector.transpose(s1t[:], s1b[:])
                    # W-direction DCT + mask: two bf16 matmuls over the
                    # strided free-column split f%8<4 (mask-scaled weights)
                    # and f%8>=4 (plain weights)
                    p2 = psum_pool.tile([P, W], F32, tag="p2")
                    p2v = p2[:].rearrange("p (g u) -> p g u", u=8)
                    s1v = s1t[:].rearrange("p (g u) -> p g u", u=8)
                    nc.tensor.matmul(
                        p2v[:, :, 0:4], wdb[:, 0:P], s1v[:, :, 0:4],
                        start=True, stop=True,
                    )
                    nc.tensor.matmul(
                        p2v[:, :, 4:8], wdb[:, P:2 * P], s1v[:, :, 4:8],
                        start=True, stop=True,
                    )
                    # block transpose back to natural layout, PSUM -> SBUF
                    s2t = work.tile([P, W], F32, tag="s2t", bufs=8)
                    nc.vector.transpose(s2t[:], p2[:])
                    # outputs ride the ACT HWDGE queue; inputs own the SP queue
                    pending = (out[b, 0, hs:hs + P, :], s2t[:])
            nc.scalar.dma_start(*pending)
    nc.compile()
    return nc


def _host_constants(dct_matrix, mask):
    D = np.asarray(dct_matrix, dtype=np.float32)
    M = np.asarray(mask, dtype=np.float32)
    kron = np.kron(np.eye(P // 8, dtype=np.float32), D.T).astype(np.float32)
    # mm1 weights carry the grayscale channel weights
    wts = np.concatenate(
        [np.float32(GRAY_W[c]) * kron for c in range(C)], axis=1
    ).astype(np.float32)
    # mm2 weights: mask zeroes (out-partition%8 < 4) only for the f%8<4
    # free columns -> scale the masked copy's columns by M[0, i%8]
    pi = np.arange(P)
    mvec = np.ascontiguousarray(M[0, pi % 8], dtype=np.float32)
    wtsb = np.concatenate([kron * mvec[None, :], kron], axis=1).astype(
        mybir.dt.np(BF16)
    )
    return wts, wtsb


def kernel(x, dct_matrix, mask):
    global _NC, LAST_RUN
    x = np.ascontiguousarray(np.asarray(x, dtype=np.float32))
    assert x.shape == (B, C, H, W)
    wts, wtsb = _host_constants(dct_matrix, mask)

    if _NC is None:
        _NC = _build_bass()

    in_maps = [
        {"x": np.ascontiguousarray(x[i * BLOC:(i + 1) * BLOC]),
         "wts": wts, "wtsb": wtsb}
        for i in range(N_CORES)
    ]
    trace = bool(int(os.environ.get("DCT_TRACE", "0")))
    LAST_RUN = run_bass_kernel_spmd(
        _NC, in_maps, list(range(N_CORES)), trace=trace,
    )
    out = np.concatenate([LAST_RUN.results[i]["out"] for i in range(N_CORES)], axis=0)
    return out


# revision 15
# speedup vs baseline: 1.0213x; 1.0213x over previous
"""Trainium2 Bass kernel for blocked-DCT high-frequency extractor.

Computes, for x (64, 3, 512, 512) f32:
  gray = 0.299*R + 0.587*G + 0.114*B                     (B,1,H,W)
  per 8x8 block:  Y = mask * (D @ block @ D.T)           (2D DCT + high-pass)
  output (64, 1, 512, 512) f32

Strategy (pure data parallel over batch, 8 batches/core on 8 cores; the
kernel is HBM-bound: 24 MiB in + 8 MiB out per core; DMA-union busy on
HW measures ~94.4 us at ~355 GB/s = the per-NC HBM cap, so the span
target is startup + ~94.4 us + a short drain).

Per core, per (batch, 128-row chunk) of the image:
  1. One fused 768 KB DMA on the SP HWDGE queue brings all 3 channel
     chunks into a (128h, 3*512w) tile (2 KB contiguous runs).
  2. H-direction DCT *and* grayscale in one PSUM accumulation group:
     three float32r matmuls (one per channel c) with stationary weight
     w_c * (I_16 kron D^T) summing into one f32 PSUM tile. float32r
     streams fp32 data through the PE at 1 cycle/row (vs 4 for plain
     f32) when the moving free dim is >= 256. This removes the whole
     3-engine grayscale stage (DVE stt + ACT mul + GpSimd add).
  3. ACT drains PSUM -> SBUF with an f32 -> bf16 round (its only
     compute op; stream transpose cannot convert dtypes).
  4. DVE 32x32-block stream transpose (2-byte). Because 8 | 32, this
     puts the intra-block w index on partitions.
  5. W-direction DCT *and* high-pass mask as two bf16 matmuls over the
     two strided free-column groups f%8<4 / f%8>=4: the mask only
     depends on (out-partition % 8 < 4) && (free % 8 < 4), so the f<4
     group uses weights whose columns are pre-scaled by the 0/1 mask
     vector and the f>=4 group uses plain weights. The mask costs zero
     extra ops anywhere.
  6. DVE stream-transpose back, f32 PSUM -> SBUF -> exact natural
     row-major output layout.
  7. 256 KB contiguous output DMA on the ACT HWDGE queue (separate
     queue from the SP input stream).

The per-chunk dependency chain crosses Tensor -> ACT -> DVE -> Tensor;
with naive emission the in-order Tensor queue would serialize on that
round trip (~3.2 us/chunk > the 2.95 us/chunk DMA pace, observed to
throttle the input stream). The emission is therefore software-
pipelined: iteration i issues mm1(i) but mm2(i-1), t2(i-1) and the
output DMA for chunk i-2, so every op's dependencies are at least one
chunk old and no engine parks. xin prefetch depth is kept at 4 chunks:
deep prefetch only lengthens the compute-paced drain after the input
stream exhausts.
"""

import os

import numpy as np

import concourse.bacc as bacc
import concourse.mybir as mybir
import concourse.tile as tile
from concourse.bass_utils import run_bass_kernel_spmd

N_CORES = 8
B, C, H, W = 64, 3, 512, 512
BLOC = B // N_CORES  # batches per core
P = 128              # SBUF partitions / chunk height
NCH = H // P         # 128-row chunks per image
NCHUNK = BLOC * NCH  # chunks per core
F32 = mybir.dt.float32
F32R = mybir.dt.float32r
BF16 = mybir.dt.bfloat16
GRAY_W = (0.299, 0.587, 0.114)

_NC = None          # cached compiled Bass module
LAST_RUN = None     # BassKernelResults of the most recent run (for test.py)


def _build_bass():
    nc = bacc.Bacc(
        "TRN2",
        target_bir_lowering=False,
        debug=False,
        num_devices=N_CORES,
    )
    x = nc.declare_dram_parameter("x", [BLOC, C, H, W], F32R, isOutput=False)
    # wts[:, 3*128]: [w0*K | w1*K | w2*K] with K = I_16 kron D^T (mm1, f32r)
    wts = nc.declare_dram_parameter("wts", [P, 3 * P], F32R, isOutput=False)
    # bf16 weights for the W-direction matmul: [mask-scaled K | plain K]
    wtsb = nc.declare_dram_parameter("wtsb", [P, 2 * P], BF16, isOutput=False)
    out = nc.declare_dram_parameter("out", [BLOC, 1, H, W], F32, isOutput=True)

    with tile.TileContext(nc) as tc:
        with (
            tc.tile_pool(name="consts", bufs=1) as consts,
            tc.tile_pool(name="xin", bufs=4) as xin,
            tc.tile_pool(name="work", bufs=4) as work,
            tc.tile_pool(name="psum", bufs=4, space="PSUM") as psum_pool,
        ):
            # consts ride the ACT HWDGE queue so the SP queue's first
            # descriptor batch is already the first 768 KB input chunk.
            wd = consts.tile([P, 3 * P], F32R, tag="wd")
            nc.scalar.dma_start(wd[:], wts[:])
            wdb = consts.tile([P, 2 * P], BF16, tag="wdb")
            nc.scalar.dma_start(wdb[:], wtsb[:])

            # software-pipelined emission: stage A ops for chunk i, stage B
            # (mm2 + t2) for chunk i-1, output DMA for chunk i-2
            p1s, s1ts, p2s, s2ts = {}, {}, {}, {}
            for i in range(NCHUNK + 2):
                if i < NCHUNK:
                    b, hc = divmod(i, NCH)
                    # one 768 KB DMA: channels side by side in the free dim
                    xt = xin.tile([P, C * W], F32R, tag="x")
                    xsrc = x[b].rearrange("c (n p) w -> n p c w", p=P)[hc]
                    nc.sync.dma_start(
                        xt[:].rearrange("p (c w) -> p c w", w=W), xsrc
                    )
                    # H-direction DCT + grayscale: 3 accumulating f32r
                    # matmuls, one per channel
                    p1 = psum_pool.tile([P, W], F32, tag="p1")
                    for c in range(C):
                        nc.tensor.matmul(
                            p1[:],
                            wd[:, c * P:(c + 1) * P],
                            xt[:, c * W:(c + 1) * W],
                            start=(c == 0),
                            stop=(c == C - 1),
                        )
                    p1s[i] = p1
                if i - 1 in s1ts:
                    # W-direction DCT + mask for chunk i-1: two bf16
                    # matmuls over the strided free-column split f%8<4
                    # (mask-scaled weights) / f%8>=4 (plain weights)
                    s1v = s1ts[i - 1][:].rearrange("p (g u) -> p g u", u=8)
                    p2 = psum_pool.tile([P, W], F32, tag="p2")
                    p2v = p2[:].rearrange("p (g u) -> p g u", u=8)
                    nc.tensor.matmul(
                        p2v[:, :, 0:4], wdb[:, 0:P], s1v[:, :, 0:4],
                        start=True, stop=True,
                    )
                    nc.tensor.matmul(
                        p2v[:, :, 4:8], wdb[:, P:2 * P], s1v[:, :, 4:8],
                        start=True, stop=True,
                    )
                    p2s[i - 1] = p2
                if i < NCHUNK:
                    # PSUM -> SBUF with bf16 rounding on ACT (its only
                    # compute op; stream transpose cannot convert dtypes)
                    s1b = work.tile([P, W], BF16, tag="s1b")
                    nc.scalar.copy(s1b[:], p1s.pop(i)[:])
                if i - 2 in s2ts:
                    # output DMA for chunk i-2 on the ACT HWDGE queue
                    bb, hh = divmod(i - 2, NCH)
                    nc.scalar.dma_start(
                        out[bb, 0, hh * P:(hh + 1) * P, :], s2ts.pop(i - 2)[:]
                    )
                if i < NCHUNK:
                    # 2-byte 32x32 block transpose
                    s1t = work.tile([P, W], BF16, tag="s1t")
                    nc.vector.transpose(s1t[:], s1b[:])
                    s1ts[i] = s1t
                if i - 1 in p2s:
                    # block transpose back to natural layout, PSUM -> SBUF
                    s2t = work.tile([P, W], F32, tag="s2t", bufs=6)
                    nc.vector.transpose(s2t[:], p2s.pop(i - 1)[:])
                    s2ts[i - 1] = s2t
            # flush the last two output DMAs
            for i in sorted(s2ts):
                bb, hh = divmod(i, NCH)
                nc.scalar.dma_start(
                    out[bb, 0, hh * P:(hh + 1) * P, :], s2ts[i][:]
                )
    nc.compile()
    return nc


def _host_constants(dct_matrix, mask):
    D = np.asarray(dct_matrix, dtype=np.float32)
    M = np.asarray(mask, dtype=np.float32)
    kron = np.kron(np.eye(P // 8, dtype=np.float32), D.T).astype(np.float32)
    # mm1 weights carry the grayscale channel weights
    wts = np.concatenate(
        [np.float32(GRAY_W[c]) * kron for c in range(C)], axis=1
    ).astype(np.float32)
    # mm2 weights: mask zeroes (out-partition%8 < 4) only for the f%8<4
    # free columns -> scale the masked copy's columns by M[0, i%8]
    pi = np.arange(P)
    mvec = np.ascontiguousarray(M[0, pi % 8], dtype=np.float32)
    wtsb = np.concatenate([kron * mvec[None, :], kron], axis=1).astype(
        mybir.dt.np(BF16)
    )
    return wts, wtsb


def kernel(x, dct_matrix, mask):
    global _NC, LAST_RUN
    x = np.ascontiguousarray(np.asarray(x, dtype=np.float32))
    assert x.shape == (B, C, H, W)
    wts, wtsb = _host_constants(dct_matrix, mask)

    if _NC is None:
        _NC = _build_bass()

    in_maps = [
        {"x": np.ascontiguousarray(x[i * BLOC:(i + 1) * BLOC]),
         "wts": wts, "wtsb": wtsb}
        for i in range(N_CORES)
    ]
    trace = bool(int(os.environ.get("DCT_TRACE", "0")))
    LAST_RUN = run_bass_kernel_spmd(
        _NC, in_maps, list(range(N_CORES)), trace=trace,
    )
    out = np.concatenate([LAST_RUN.results[i]["out"] for i in range(N_CORES)], axis=0)
    return out
